# revision 50
# baseline (speedup 1.0000x reference)
"""BernConv (K=2) GNN message passing on 8 Trainium2 NeuronCores.

Self-contained kernel: kernel(**inputs) -> np.ndarray [N, 32] float32.

Strategy (v2, ap_gather): keep the whole fp16 feature table SBUF-resident
per source-chunk and gather src rows with the GPSIMD `ap_gather` SBUF
gather (the dma_gather/HBM path is descriptor-rate bound at ~60 GB/s
effective for 256B random reads).

Layout: nodes dealt (by degree) over 8 cores x 8 GPSIMD groups x SLOTS
slots; feature dim pairs (2l, 2l+1) live on partition lane l of each
16-partition group, i.e. every group holds all 32 dims at d=2 fp16 per
node. The gather table [128, NLOC*2] fp16 (row c*16+l = core c, lane l)
is AllGathered between rounds; each round every core streams the table
in NCHUNK=4 source chunks into a [128, CH*2] SBUF tile (8 group
replicas) and runs per-group ap_gathers + bucketed DVE segment reduces:
    g0 = dh*feat;  g_k = g_{k-1} + dh^2 * agg(g_{k-1})   (k = 1, 2)
    out = s1 * dh^-1 * g2 - s2 * dh * agg(g2)
"""
import sys
sys.path.insert(0, "/opt/trn_rl_repo")

import numpy as np
import concourse.bacc as bacc
import concourse.mybir as mybir
import concourse.tile as tile
from concourse import bass_utils

NC = 8
G = 8
P = 128
D = 32
SLOTS = 784              # last slot (783) reserved as guaranteed-zero
NLOC = G * SLOTS         # 6272
NPAD = NC * NLOC         # 50176
NCHUNK = 4
CH = NPAD // NCHUNK      # 12544
ZIDX = CH - 1            # rel row of core 2k+1, g=7, s=783 -> reserved zero
JMAX_RAW = 2560          # target segment size (pre %16 pad)

F32 = mybir.dt.float32
F16 = mybir.dt.float16
I16 = mybir.dt.int16


# --------------------------------------------------------------------------
# host-side layout
# --------------------------------------------------------------------------

def choose_levels(req, max_levels=14):
    """Bucket levels minimizing total padded degree (DP on req histogram)."""
    Lmax = int(req.max())
    hist = np.bincount(req, minlength=Lmax + 1).astype(np.int64)
    cnt_le = hist.cumsum()
    INF = float("inf")

    def interval_cost(a, b):
        return int(cnt_le[b] - cnt_le[a]) * b

    f = np.full((max_levels + 1, Lmax + 1), INF)
    prev = np.zeros((max_levels + 1, Lmax + 1), dtype=np.int64)
    f[0, 0] = 0.0
    for m in range(1, max_levels + 1):
        for b in range(1, Lmax + 1):
            best, besta = INF, 0
            for a in range(0, b):
                if f[m - 1, a] == INF:
                    continue
                c = f[m - 1, a] + interval_cost(a, b)
                if c < best:
                    best, besta = c, a
            f[m, b] = best
            prev[m, b] = besta
    m_best = int(np.argmin(f[:, Lmax]))
    levels = []
    b, m = Lmax, m_best
    while b > 0:
        levels.append(b)
        b = int(prev[m, b])
        m -= 1
    return np.array(sorted(levels), dtype=np.int64)


def balance_assign(src, dst, deg, n_nodes):
    """Assign nodes to the 64 (c,g) groups, batch-dealing by degree (keeps
    group sizes and degree profiles aligned) while greedily choosing each
    node's CHUNK to flatten its out-neighbors' per-chunk in-edge counts
    (cuts the bucket padding that is driven by max-over-chunks)."""
    order = np.argsort(-deg, kind="stable")
    # out-adjacency sorted by src
    eo = np.argsort(src, kind="stable")
    s_src = src[eo]
    s_dst = dst[eo]
    out_start = np.searchsorted(s_src, np.arange(n_nodes))
    out_end = np.searchsorted(s_src, np.arange(n_nodes), side="right")

    cnt = np.zeros((n_nodes, NCHUNK), dtype=np.int32)
    curmax = np.zeros(n_nodes, dtype=np.int32)
    node_cg = np.full(n_nodes, -1, dtype=np.int64)
    gpc = G * (NC // NCHUNK)          # groups per chunk (16)
    nb = (n_nodes + 63) // 64

    def sweep():
        # rotate group fill order per batch so profiles stay aligned
        for b in range(nb):
            batch = order[b * 64:(b + 1) * 64]
            nbrs = [s_dst[out_start[i]:out_end[i]] for i in batch]
            # remove current contribution (refinement passes)
            for j, i in enumerate(batch):
                if node_cg[i] >= 0:
                    k_old = (node_cg[i] // G) // (NC // NCHUNK)
                    chunk_total[k_old] -= 1
                    if len(nbrs[j]):
                        np.add.at(cnt, (nbrs[j], k_old), -1)
                        curmax[nbrs[j]] = cnt[nbrs[j]].max(axis=1)
            costs = np.zeros((len(batch), NCHUNK), dtype=np.int64)
            for j, nb_j in enumerate(nbrs):
                if len(nb_j):
                    m = curmax[nb_j]
                    c = cnt[nb_j]
                    costs[j] = (2 * np.maximum(c + 1 - m[:, None], 0)
                                + (c + 1 == m[:, None])).sum(axis=0)
            # soft per-batch quota (2x the fair share) + hard global cap
            cap = np.full(NCHUNK, 2 * gpc, dtype=np.int64)
            slot_in_chunk = np.zeros(NCHUNK, dtype=np.int64)
            sc = np.sort(costs, axis=1)
            regret = sc[:, 1] - sc[:, 0] if NCHUNK > 1 else sc[:, 0]
            for j in np.argsort(-regret):
                ks = np.argsort(costs[j], kind="stable")
                k = next(k for k in ks
                         if cap[k] > 0 and chunk_total[k] < CHCAP)
                cap[k] -= 1
                chunk_total[k] += 1
                i = batch[j]
                g_off = (b + slot_in_chunk[k]) % gpc
                node_cg[i] = (k * (NC // NCHUNK) + g_off // G) * G + (g_off % G)
                slot_in_chunk[k] += 1
                nb_j = nbrs[j]
                if len(nb_j):
                    np.add.at(cnt, (nb_j, k), 1)
                    curmax[nb_j] = np.maximum(curmax[nb_j], cnt[nb_j, k])

    CHCAP = gpc * (SLOTS - 1) - 16
    chunk_total = np.zeros(NCHUNK, dtype=np.int64)
    for _ in range(5):
        sweep()
    return node_cg


def build_layout(src, dst, n_nodes):
    E = src.shape[0]
    deg = np.bincount(dst, minlength=n_nodes).astype(np.int64)

    node_cg = balance_assign(src, dst, deg, n_nodes)
    node_core = node_cg // G
    node_grp = node_cg % G
    node_chunk = node_core // (NC // NCHUNK)   # chunk of a node as src

    cnt = np.zeros((n_nodes, NCHUNK), dtype=np.int64)
    np.add.at(cnt, (dst, node_chunk[src]), 1)
    req = np.maximum(cnt.max(axis=1), 1)

    levels = choose_levels(req)
    ghat = levels[np.searchsorted(levels, req)]

    # re-deal within each chunk by ghat rank: the 16 groups of a chunk get
    # exactly aligned level profiles, so the shared template (max over all
    # 64 groups) is just the elementwise max of the 4 chunk profiles.
    gpc = G * (NC // NCHUNK)
    node_core = np.empty(n_nodes, dtype=np.int64)
    node_grp = np.empty(n_nodes, dtype=np.int64)
    node_slot = np.empty(n_nodes, dtype=np.int64)
    tmpl = np.zeros(SLOTS - 1, dtype=np.int64)
    for kchunk in range(NCHUNK):
        nk = np.where(node_chunk == kchunk)[0]
        nk = nk[np.lexsort((nk, -ghat[nk]))]
        r = np.arange(len(nk))
        go = r % gpc
        node_core[nk] = 2 * kchunk + go // G
        node_grp[nk] = go % G
        node_slot[nk] = r // gpc
        prof = ghat[nk][::gpc]            # rank-0-of-each-slot = profile max
        assert len(prof) <= SLOTS - 1, len(prof)
        tmpl[: len(prof)] = np.maximum(tmpl[: len(prof)], prof)
    node_cg = node_core * G + node_grp
    nslots_used = int((tmpl > 0).sum())
    tmpl = tmpl[:nslots_used]
    prefixB = np.concatenate([[0], np.cumsum(tmpl)])
    TJ = int(prefixB[-1])

    # segments: greedy cut at slot boundaries, each <= JMAX_RAW raw idxs
    seg_bounds = [0]
    for s in range(nslots_used):
        if prefixB[s + 1] - prefixB[seg_bounds[-1]] > JMAX_RAW:
            seg_bounds.append(s)
    seg_bounds.append(nslots_used)
    segs = []
    for i in range(len(seg_bounds) - 1):
        lo, hi = seg_bounds[i], seg_bounds[i + 1]
        raw = int(prefixB[hi] - prefixB[lo])
        # %32 (not %16): keeps every idx-slice base 4-byte aligned — the
        # gather ucode reads idxs as 32-bit words and a 2-byte-misaligned
        # AP base corrupts every 4th word's high half.
        pad = (-raw) % 32
        segs.append((lo, hi, raw, raw + pad))
    JMAXP = max(s[3] for s in segs)
    TPJ = sum(s[3] for s in segs)

    # runs of equal level inside each segment
    runs = []
    for si, (lo, hi, raw, padded) in enumerate(segs):
        s = lo
        off = 0
        while s < hi:
            b = int(tmpl[s])
            e = s
            while e < hi and tmpl[e] == b:
                e += 1
            runs.append((si, off, e - s, b, s))
            off += (e - s) * b
            s = e

    # fill per (c,g,chunk) idx arrays with ZIDX pads
    pi = node_core * NLOC + node_grp * SLOTS + node_slot
    src_row = pi[src]
    e_chunk = src_row // CH
    e_rel = src_row - e_chunk * CH

    okey_e = np.lexsort((e_rel, e_chunk, dst))
    s_dst = dst[okey_e]
    s_chunk = e_chunk[okey_e]
    s_rel = e_rel[okey_e]
    key = s_dst * NCHUNK + s_chunk
    run_start = np.searchsorted(key, key)
    erank = np.arange(E) - run_start
    assert (erank < ghat[s_dst]).all()

    A = np.full((NC * G, NCHUNK, TJ), ZIDX, dtype=np.int16)
    flat_pos = (node_cg[s_dst] * NCHUNK + s_chunk) * TJ \
        + prefixB[node_slot[s_dst]] + erank
    A.reshape(-1)[flat_pos] = s_rel.astype(np.int16)

    parts = []
    for (lo, hi, raw, padded) in segs:
        blk = A[:, :, prefixB[lo]:prefixB[hi]]
        if padded > raw:
            padblk = np.full((NC * G, NCHUNK, padded - raw), ZIDX, np.int16)
            blk = np.concatenate([blk, padblk], axis=2)
        parts.append(blk)
    AP_ = np.concatenate(parts, axis=2)

    return dict(
        deg=deg, node_cg=node_cg, node_core=node_core, node_grp=node_grp,
        node_slot=node_slot, pi=pi, tmpl=tmpl, prefixB=prefixB,
        segs=segs, runs=runs, TJ=TJ, TPJ=TPJ, JMAXP=JMAXP,
        nslots_used=nslots_used, idx=AP_, levels=levels,
    )


def make_host_data(feat, weight, src, dst):
    n_nodes = feat.shape[0]
    lay = build_layout(src, dst, n_nodes)

    deg = lay["deg"].astype(np.float64)
    degc = np.maximum(deg, 1.0).astype(np.float32)
    dh = (degc ** -0.5).astype(np.float32)
    dh2 = dh * dh
    dhinv = 1.0 / dh

    node_core = lay["node_core"]
    node_grp = lay["node_grp"]
    node_slot = lay["node_slot"]
    nodes = np.arange(n_nodes)

    def slot_tile(vals):
        t = np.zeros((NC, P, SLOTS), np.float32)
        for l in range(16):
            t[node_core, node_grp * 16 + l, node_slot] = vals
        return t

    dh2l = slot_tile(dh2)
    dhhl = slot_tile(dh)
    dhil = slot_tile(dhinv)

    g0v = feat * dh[:, None]
    g0slot = np.zeros((NC, P, SLOTS, 2), np.float32)
    g0tab = np.zeros((P, NLOC, 2), np.float16)
    for l in range(16):
        g0slot[node_core, node_grp * 16 + l, node_slot, 0] = g0v[nodes, 2 * l]
        g0slot[node_core, node_grp * 16 + l, node_slot, 1] = g0v[nodes, 2 * l + 1]
        g0tab[node_core * 16 + l, node_grp * SLOTS + node_slot, 0] = \
            g0v[nodes, 2 * l].astype(np.float16)
        g0tab[node_core * 16 + l, node_grp * SLOTS + node_slot, 1] = \
            g0v[nodes, 2 * l + 1].astype(np.float16)
    g0slot = g0slot.reshape(NC, P, SLOTS * 2)
    g0tab = g0tab.reshape(P, NLOC * 2)
    # pre-replicated round-1 chunk tables (input upload is free)
    g0rep = []
    for kc in range(NCHUNK):
        rows = g0tab[2 * kc * 16:(2 * kc + 2) * 16].reshape(2, 16, NLOC * 2)
        blk = np.concatenate([rows[0], rows[1]], axis=1)      # [16, CH*2]
        g0rep.append(np.ascontiguousarray(np.tile(blk, (G, 1))))

    idx = lay["idx"].reshape(NC, G, NCHUNK * lay["TPJ"])
    idxw = idx.reshape(NC, G, -1, 16).transpose(0, 1, 3, 2)
    idxw = np.ascontiguousarray(idxw.reshape(NC, P, -1))

    coef = np.array([[0.25, 0.5, 0.25, 0.0, 0.5, 0.25]], np.float32)
    w2 = np.ascontiguousarray(weight.reshape(1, 3).astype(np.float32))

    per_core = []
    for c in range(NC):
        pc = dict(
            g0tab=g0tab, g0slot=np.ascontiguousarray(g0slot[c]),
            idx=idxw[c], dh2l=np.ascontiguousarray(dh2l[c]),
            dhhl=np.ascontiguousarray(dhhl[c]),
            dhil=np.ascontiguousarray(dhil[c]),
            w=w2, coef=coef,
        )
        for kc in range(NCHUNK):
            pc[f"g0rep{kc}"] = g0rep[kc]
        per_core.append(pc)
    return lay, per_core


def assemble_output(lay, outs, n_nodes):
    full = np.stack(outs)                          # [NC, 128, SLOTS*2]
    full = full.reshape(NC, G, 16, SLOTS, 2)
    res = np.empty((n_nodes, D), np.float32)
    nc_, ng, ns = lay["node_core"], lay["node_grp"], lay["node_slot"]
    for l in range(16):
        res[:, 2 * l] = full[nc_, ng, l, ns, 0]
        res[:, 2 * l + 1] = full[nc_, ng, l, ns, 1]
    return res


# --------------------------------------------------------------------------
# device kernel
# --------------------------------------------------------------------------

def build_kernel(segs, runs, TPJ, JMAXP):
    TOTC = NCHUNK * TPJ // 16     # idx cols per partition
    S2 = SLOTS * 2

    nc = bacc.Bacc("TRN2", target_bir_lowering=False)
    g0rep_d = [nc.dram_tensor(f"g0rep{kc}", [P, CH * 2], F16,
                              kind="ExternalInput") for kc in range(NCHUNK)]
    g0slot_d = nc.dram_tensor("g0slot", [P, S2], F32, kind="ExternalInput")
    idx_d = nc.dram_tensor("idx", [P, TOTC], I16, kind="ExternalInput")
    dh2l_d = nc.dram_tensor("dh2l", [P, SLOTS], F32, kind="ExternalInput")
    dhhl_d = nc.dram_tensor("dhhl", [P, SLOTS], F32, kind="ExternalInput")
    dhil_d = nc.dram_tensor("dhil", [P, SLOTS], F32, kind="ExternalInput")
    w_d = nc.dram_tensor("w", [1, 3], F32, kind="ExternalInput")
    coef_d = nc.dram_tensor("coef", [1, 6], F32, kind="ExternalInput")
    out_d = nc.dram_tensor("out", [P, S2], F32, kind="ExternalOutput")

    seg_off = [0]
    for s in segs:
        seg_off.append(seg_off[-1] + s[3])

    with tile.TileContext(nc) as tc:
        with (
            tc.tile_pool(name="dram", bufs=1, space="DRAM") as dramp,
            tc.tile_pool(name="persist", bufs=1) as persist,
            tc.tile_pool(name="tblp", bufs=2) as tblp,
            tc.tile_pool(name="slabp", bufs=3) as slabp,
        ):
            gtabs = [dramp.tile([P, NLOC * 2], F16, name=f"gtab{k}",
                                addr_space="Shared")
                     for k in range(2)]
            bounces = [dramp.tile([16, NLOC * 2], F16, name=f"bnc{k}")
                       for k in range(2)]

            idxt = persist.tile([P, TOTC], I16, name="idxt")
            nc.sync.dma_start(out=idxt[:], in_=idx_d[:])
            dh2l = persist.tile([P, SLOTS], F32, name="dh2l")
            dhhl = persist.tile([P, SLOTS], F32, name="dhhl")
            dhil = persist.tile([P, SLOTS], F32, name="dhil")
            nc.sync.dma_start(out=dh2l[:], in_=dh2l_d[:])
            nc.sync.dma_start(out=dhhl[:], in_=dhhl_d[:])
            nc.sync.dma_start(out=dhil[:], in_=dhil_d[:])

            # scalars s1, s2
            wt = persist.tile([1, 3], F32, name="wt")
            coefs = persist.tile([1, 6], F32, name="coefs")
            nc.sync.dma_start(out=wt[:], in_=w_d[:])
            nc.sync.dma_start(out=coefs[:], in_=coef_d[:])
            wr = persist.tile([1, 3], F32, name="wr")
            nc.vector.tensor_scalar(out=wr[:], in0=wt[:], scalar1=0.0,
                                    scalar2=None, op0=mybir.AluOpType.max)
            sprod = persist.tile([1, 6], F32, name="sprod")
            nc.vector.tensor_tensor(out=sprod[:, 0:3], in0=wr[:],
                                    in1=coefs[:, 0:3], op=mybir.AluOpType.mult)
            nc.vector.tensor_tensor(out=sprod[:, 3:6], in0=wr[:],
                                    in1=coefs[:, 3:6], op=mybir.AluOpType.mult)
            svals = persist.tile([1, 2], F32, name="svals")
            nc.vector.tensor_reduce(out=svals[:, 0:1], in_=sprod[:, 0:3],
                                    axis=mybir.AxisListType.X,
                                    op=mybir.AluOpType.add)
            nc.vector.tensor_reduce(out=svals[:, 1:2], in_=sprod[:, 3:6],
                                    axis=mybir.AxisListType.X,
                                    op=mybir.AluOpType.add)
            sbc = persist.tile([P, 2], F32, name="sbc")
            nc.gpsimd.partition_broadcast(sbc[:], svals[:])
            # fold s1/s2 into the final-combine scale tiles up front so the
            # sliced final round needs no tensor_scalar (2-port) ops
            nc.vector.tensor_scalar(out=dhil[:], in0=dhil[:],
                                    scalar1=sbc[:, 0:1], scalar2=None,
                                    op0=mybir.AluOpType.mult)
            nc.vector.tensor_scalar(out=dhhl[:], in0=dhhl[:],
                                    scalar1=sbc[:, 1:2], scalar2=None,
                                    op0=mybir.AluOpType.mult)

            zt = persist.tile([P, 1], F32, name="zt")
            nc.gpsimd.memset(zt[:], 0.0)

            gA = persist.tile([P, S2], F32, name="gA")
            gB = persist.tile([P, S2], F32, name="gB")
            nc.sync.dma_start(out=gA[:], in_=g0slot_d[:])

            partials = [persist.tile([P, S2], F32, name=f"part{kc}")
                        for kc in range(NCHUNK)]
            for t in partials:
                nc.gpsimd.memset(t[:], 0.0)

            agg = persist.tile([P, S2], F32, name="agg")
            ut = persist.tile([P, S2], F32, name="ut")
            tt = persist.tile([P, S2], F32, name="tt")
            shipt = [persist.tile([P, S2], F16, name=f"shipt{i}")
                     for i in range(2)]

            for k in (1, 2, 3):
                slab0 = None
                for kc in range(NCHUNK):
                    tbl = tblp.tile([P, CH * 2], F16, tag="tbl",
                                    name=f"tbl{k}_{kc}")
                    if kc == 1 and slab0 is not None:
                        # order-pin: chunk-1 loads WAW on this corner, which
                        # depends on the chunk-0 first gather — stops the
                        # scheduler enqueueing c1 loads before that gather's
                        # sem waits (which use cumulative lane thresholds)
                        nc.vector.tensor_tensor(
                            out=tbl[:, 0:2], in0=slab0[:, 0:2],
                            in1=slab0[:, 0:2], op=mybir.AluOpType.mult)
                    if k == 1:
                        # round 1: host pre-replicated table, one flat DMA
                        nc.sync.dma_start(out=tbl[:], in_=g0rep_d[kc][:])
                    else:
                        tabsrc = gtabs[k - 2]
                        src32 = tabsrc[2 * kc * 16:(2 * kc + 2) * 16, :] \
                            .rearrange("(h l) f -> l h f", h=2)
                        for g in range(G):
                            nc.sync.dma_start(
                                out=tbl[g * 16:(g + 1) * 16, :].rearrange(
                                    "l (h f) -> l h f", h=2),
                                in_=src32)
                    for si, (lo, hi, raw, padded) in enumerate(segs):
                        slab = slabp.tile([P, JMAXP * 2], F16, tag="slab",
                                          name=f"slab{k}_{kc}_{si}")
                        if kc == 0 and si == 0:
                            slab0 = slab
                        base16 = (kc * TPJ + seg_off[si]) // 16
                        nc.gpsimd.ap_gather(
                            out_ap=slab[:, 0:padded * 2].rearrange(
                                "p (n d) -> p n d", d=2),
                            in_ap=tbl[:].rearrange("p (n d) -> p n d", d=2),
                            idxs_ap=idxt[:, base16:base16 + padded // 16],
                            channels=P, num_elems=CH, d=2, num_idxs=padded)
                        for (si2, off, R, b, s0) in runs:
                            if si2 != si:
                                continue
                            nc.vector.tensor_reduce(
                                out=partials[kc][:, s0 * 2:(s0 + R) * 2]
                                .rearrange("p (r j) -> p r j", j=2),
                                in_=slab[:, off * 2:(off + R * b) * 2]
                                .rearrange("p (r b j) -> p r j b", b=b, j=2),
                                axis=mybir.AxisListType.X,
                                op=mybir.AluOpType.add)
                        if kc == NCHUNK - 1:
                            # slot-sliced round tail overlaps the remaining
                            # gathers of the last chunk
                            gold = gA if k != 2 else gB
                            gnew = gB if k == 1 else gA
                            lo2 = lo * 2
                            hi2 = hi * 2 if si < len(segs) - 1 else S2
                            ns = (hi2 - lo2) // 2
                            nc.vector.tensor_tensor(
                                out=agg[:, lo2:hi2],
                                in0=partials[0][:, lo2:hi2],
                                in1=partials[1][:, lo2:hi2],
                                op=mybir.AluOpType.add)
                            nc.vector.tensor_tensor(
                                out=agg[:, lo2:hi2], in0=agg[:, lo2:hi2],
                                in1=partials[2][:, lo2:hi2],
                                op=mybir.AluOpType.add)
                            nc.vector.tensor_tensor(
                                out=agg[:, lo2:hi2], in0=agg[:, lo2:hi2],
                                in1=partials[3][:, lo2:hi2],
                                op=mybir.AluOpType.add)
                            if k < 3:
                                nc.vector.tensor_tensor(
                                    out=tt[:, lo2:hi2].rearrange(
                                        "p (s j) -> p s j", j=2),
                                    in0=agg[:, lo2:hi2].rearrange(
                                        "p (s j) -> p s j", j=2),
                                    in1=dh2l[:, lo2 // 2:hi2 // 2]
                                    .to_broadcast([P, ns, 2]),
                                    op=mybir.AluOpType.mult)
                                nc.vector.tensor_tensor(
                                    out=gnew[:, lo2:hi2], in0=tt[:, lo2:hi2],
                                    in1=gold[:, lo2:hi2],
                                    op=mybir.AluOpType.add)
                                sh = shipt[k - 1]
                                nc.vector.tensor_tensor(
                                    out=sh[:, lo2:hi2], in0=gnew[:, lo2:hi2],
                                    in1=zt[:].to_broadcast([P, hi2 - lo2]),
                                    op=mybir.AluOpType.add)
                                for g in range(G):
                                    nc.sync.dma_start(
                                        out=bounces[k - 1][
                                            :, g * S2 + lo2:g * S2 + hi2],
                                        in_=sh[g * 16:(g + 1) * 16, lo2:hi2])
                            else:
                                # out = s1*dhil*g2 - s2*dhhl*agg (s1, s2
                                # pre-folded into dhil/dhhl)
                                nc.vector.tensor_tensor(
                                    out=ut[:, lo2:hi2].rearrange(
                                        "p (s j) -> p s j", j=2),
                                    in0=gold[:, lo2:hi2].rearrange(
                                        "p (s j) -> p s j", j=2),
                                    in1=dhil[:, lo2 // 2:hi2 // 2]
                                    .to_broadcast([P, ns, 2]),
                                    op=mybir.AluOpType.mult)
                                nc.vector.tensor_tensor(
                                    out=tt[:, lo2:hi2].rearrange(
                                        "p (s j) -> p s j", j=2),
                                    in0=agg[:, lo2:hi2].rearrange(
                                        "p (s j) -> p s j", j=2),
                                    in1=dhhl[:, lo2 // 2:hi2 // 2]
                                    .to_broadcast([P, ns, 2]),
                                    op=mybir.AluOpType.mult)
                                nc.vector.tensor_tensor(
                                    out=ut[:, lo2:hi2], in0=ut[:, lo2:hi2],
                                    in1=tt[:, lo2:hi2],
                                    op=mybir.AluOpType.subtract)
                                nc.sync.dma_start(out=out_d[:, lo2:hi2],
                                                  in_=ut[:, lo2:hi2])

                if k < 3:
                    nc.gpsimd.collective_compute(
                        "AllGather", mybir.AluOpType.bypass,
                        replica_groups=[list(range(NC))],
                        ins=[bounces[k - 1].opt()], outs=[gtabs[k - 1].opt()])
    nc.compile()
    return nc


_CACHE = {}


def kernel(feat, weight, src, dst):
    feat = np.ascontiguousarray(np.asarray(feat, dtype=np.float32))
    weight = np.ascontiguousarray(np.asarray(weight, dtype=np.float32))
    src64 = np.asarray(src).astype(np.int64)
    dst64 = np.asarray(dst).astype(np.int64)
    n_nodes = feat.shape[0]

    lay, per_core = make_host_data(feat, weight, src64, dst64)
    key = (tuple(lay["segs"]), tuple(lay["runs"]), lay["TPJ"], lay["JMAXP"])
    if key not in _CACHE:
        _CACHE[key] = build_kernel(lay["segs"], lay["runs"], lay["TPJ"],
                                   lay["JMAXP"])
    nc = _CACHE[key]

    keys = ["g0slot", "idx", "dh2l", "dhhl", "dhil", "w", "coef"] + \
        [f"g0rep{kc}" for kc in range(NCHUNK)]
    in_maps = [{k: pc[k] for k in keys} for pc in per_core]
    res = bass_utils.run_bass_kernel_spmd(nc, in_maps, core_ids=list(range(NC)))
    outs = [res.results[c]["out"] for c in range(NC)]
    return assemble_output(lay, outs, n_nodes)


# revision 51
# speedup vs baseline: 1.1763x; 1.1763x over previous
"""BernConv (K=2) GNN message passing on 8 Trainium2 NeuronCores.

Self-contained kernel: kernel(**inputs) -> np.ndarray [N, 32] float32.

Strategy (v2, ap_gather): keep the whole fp16 feature table SBUF-resident
per source-chunk and gather src rows with the GPSIMD `ap_gather` SBUF
gather (the dma_gather/HBM path is descriptor-rate bound at ~60 GB/s
effective for 256B random reads).

Layout: nodes dealt (by degree) over 8 cores x 8 GPSIMD groups x SLOTS
slots; feature dim pairs (2l, 2l+1) live on partition lane l of each
16-partition group, i.e. every group holds all 32 dims at d=2 fp16 per
node. The gather table [128, NLOC*2] fp16 (row c*16+l = core c, lane l)
is AllGathered between rounds; each round every core streams the table
in NCHUNK=4 source chunks into a [128, CH*2] SBUF tile (8 group
replicas) and runs per-group ap_gathers + bucketed DVE segment reduces:
    g0 = dh*feat;  g_k = g_{k-1} + dh^2 * agg(g_{k-1})   (k = 1, 2)
    out = s1 * dh^-1 * g2 - s2 * dh * agg(g2)
"""
import sys
sys.path.insert(0, "/opt/trn_rl_repo")

import numpy as np
import concourse.bacc as bacc
import concourse.mybir as mybir
import concourse.tile as tile
from concourse import bass_utils

NC = 8
G = 8
P = 128
D = 32
SLOTS = 784              # last slot (783) reserved as guaranteed-zero
NLOC = G * SLOTS         # 6272
NPAD = NC * NLOC         # 50176
NCHUNK = 4
CH = NPAD // NCHUNK      # 12544
ZIDX = CH - 1            # rel row of core 2k+1, g=7, s=783 -> reserved zero
JMAX_RAW = 1920          # target segment size (pre %16 pad)

F32 = mybir.dt.float32
F16 = mybir.dt.float16
I16 = mybir.dt.int16


# --------------------------------------------------------------------------
# host-side layout
# --------------------------------------------------------------------------

def choose_levels(req, max_levels=14):
    """Bucket levels minimizing total padded degree (DP on req histogram)."""
    Lmax = int(req.max())
    hist = np.bincount(req, minlength=Lmax + 1).astype(np.int64)
    cnt_le = hist.cumsum()
    INF = float("inf")

    def interval_cost(a, b):
        return int(cnt_le[b] - cnt_le[a]) * b

    f = np.full((max_levels + 1, Lmax + 1), INF)
    prev = np.zeros((max_levels + 1, Lmax + 1), dtype=np.int64)
    f[0, 0] = 0.0
    for m in range(1, max_levels + 1):
        for b in range(1, Lmax + 1):
            best, besta = INF, 0
            for a in range(0, b):
                if f[m - 1, a] == INF:
                    continue
                c = f[m - 1, a] + interval_cost(a, b)
                if c < best:
                    best, besta = c, a
            f[m, b] = best
            prev[m, b] = besta
    m_best = int(np.argmin(f[:, Lmax]))
    levels = []
    b, m = Lmax, m_best
    while b > 0:
        levels.append(b)
        b = int(prev[m, b])
        m -= 1
    return np.array(sorted(levels), dtype=np.int64)


def balance_assign(src, dst, deg, n_nodes):
    """Assign nodes to the 64 (c,g) groups, batch-dealing by degree (keeps
    group sizes and degree profiles aligned) while greedily choosing each
    node's CHUNK to flatten its out-neighbors' per-chunk in-edge counts
    (cuts the bucket padding that is driven by max-over-chunks)."""
    order = np.argsort(-deg, kind="stable")
    # out-adjacency sorted by src
    eo = np.argsort(src, kind="stable")
    s_src = src[eo]
    s_dst = dst[eo]
    out_start = np.searchsorted(s_src, np.arange(n_nodes))
    out_end = np.searchsorted(s_src, np.arange(n_nodes), side="right")

    cnt = np.zeros((n_nodes, NCHUNK), dtype=np.int32)
    curmax = np.zeros(n_nodes, dtype=np.int32)
    node_cg = np.full(n_nodes, -1, dtype=np.int64)
    gpc = G * (NC // NCHUNK)          # groups per chunk (16)
    nb = (n_nodes + 63) // 64

    def sweep():
        # rotate group fill order per batch so profiles stay aligned
        for b in range(nb):
            batch = order[b * 64:(b + 1) * 64]
            nbrs = [s_dst[out_start[i]:out_end[i]] for i in batch]
            # remove current contribution (refinement passes)
            for j, i in enumerate(batch):
                if node_cg[i] >= 0:
                    k_old = (node_cg[i] // G) // (NC // NCHUNK)
                    chunk_total[k_old] -= 1
                    if len(nbrs[j]):
                        np.add.at(cnt, (nbrs[j], k_old), -1)
                        curmax[nbrs[j]] = cnt[nbrs[j]].max(axis=1)
            costs = np.zeros((len(batch), NCHUNK), dtype=np.int64)
            for j, nb_j in enumerate(nbrs):
                if len(nb_j):
                    m = curmax[nb_j]
                    c = cnt[nb_j]
                    costs[j] = (2 * np.maximum(c + 1 - m[:, None], 0)
                                + (c + 1 == m[:, None])).sum(axis=0)
            # soft per-batch quota (2x the fair share) + hard global cap
            cap = np.full(NCHUNK, 2 * gpc, dtype=np.int64)
            slot_in_chunk = np.zeros(NCHUNK, dtype=np.int64)
            sc = np.sort(costs, axis=1)
            regret = sc[:, 1] - sc[:, 0] if NCHUNK > 1 else sc[:, 0]
            for j in np.argsort(-regret):
                ks = np.argsort(costs[j], kind="stable")
                k = next(k for k in ks
                         if cap[k] > 0 and chunk_total[k] < CHCAP)
                cap[k] -= 1
                chunk_total[k] += 1
                i = batch[j]
                g_off = (b + slot_in_chunk[k]) % gpc
                node_cg[i] = (k * (NC // NCHUNK) + g_off // G) * G + (g_off % G)
                slot_in_chunk[k] += 1
                nb_j = nbrs[j]
                if len(nb_j):
                    np.add.at(cnt, (nb_j, k), 1)
                    curmax[nb_j] = np.maximum(curmax[nb_j], cnt[nb_j, k])

    CHCAP = gpc * (SLOTS - 1) - 16
    chunk_total = np.zeros(NCHUNK, dtype=np.int64)
    for _ in range(5):
        sweep()
    return node_cg


def build_layout(src, dst, n_nodes):
    E = src.shape[0]
    deg = np.bincount(dst, minlength=n_nodes).astype(np.int64)

    node_cg = balance_assign(src, dst, deg, n_nodes)
    node_core = node_cg // G
    node_grp = node_cg % G
    node_chunk = node_core // (NC // NCHUNK)   # chunk of a node as src

    cnt = np.zeros((n_nodes, NCHUNK), dtype=np.int64)
    np.add.at(cnt, (dst, node_chunk[src]), 1)
    req = np.maximum(cnt.max(axis=1), 1)

    levels = choose_levels(req)
    ghat = levels[np.searchsorted(levels, req)]

    # re-deal within each chunk by ghat rank: the 16 groups of a chunk get
    # exactly aligned level profiles, so the shared template (max over all
    # 64 groups) is just the elementwise max of the 4 chunk profiles.
    gpc = G * (NC // NCHUNK)
    node_core = np.empty(n_nodes, dtype=np.int64)
    node_grp = np.empty(n_nodes, dtype=np.int64)
    node_slot = np.empty(n_nodes, dtype=np.int64)
    tmpl = np.zeros(SLOTS - 1, dtype=np.int64)
    for kchunk in range(NCHUNK):
        nk = np.where(node_chunk == kchunk)[0]
        nk = nk[np.lexsort((nk, -ghat[nk]))]
        r = np.arange(len(nk))
        go = r % gpc
        node_core[nk] = 2 * kchunk + go // G
        node_grp[nk] = go % G
        node_slot[nk] = r // gpc
        prof = ghat[nk][::gpc]            # rank-0-of-each-slot = profile max
        assert len(prof) <= SLOTS - 1, len(prof)
        tmpl[: len(prof)] = np.maximum(tmpl[: len(prof)], prof)
    node_cg = node_core * G + node_grp
    nslots_used = int((tmpl > 0).sum())
    tmpl = tmpl[:nslots_used]
    prefixB = np.concatenate([[0], np.cumsum(tmpl)])
    TJ = int(prefixB[-1])

    # segments: greedy cut at slot boundaries, each <= JMAX_RAW raw idxs
    seg_bounds = [0]
    for s in range(nslots_used):
        if prefixB[s + 1] - prefixB[seg_bounds[-1]] > JMAX_RAW:
            seg_bounds.append(s)
    seg_bounds.append(nslots_used)
    segs = []
    for i in range(len(seg_bounds) - 1):
        lo, hi = seg_bounds[i], seg_bounds[i + 1]
        raw = int(prefixB[hi] - prefixB[lo])
        # %32 (not %16): keeps every idx-slice base 4-byte aligned — the
        # gather ucode reads idxs as 32-bit words and a 2-byte-misaligned
        # AP base corrupts every 4th word's high half.
        pad = (-raw) % 32
        segs.append((lo, hi, raw, raw + pad))
    JMAXP = max(s[3] for s in segs)
    TPJ = sum(s[3] for s in segs)

    # runs of equal level inside each segment
    runs = []
    for si, (lo, hi, raw, padded) in enumerate(segs):
        s = lo
        off = 0
        while s < hi:
            b = int(tmpl[s])
            e = s
            while e < hi and tmpl[e] == b:
                e += 1
            runs.append((si, off, e - s, b, s))
            off += (e - s) * b
            s = e

    # fill per (c,g,chunk) idx arrays with ZIDX pads
    pi = node_core * NLOC + node_grp * SLOTS + node_slot
    src_row = pi[src]
    e_chunk = src_row // CH
    e_rel = src_row - e_chunk * CH

    okey_e = np.lexsort((e_rel, e_chunk, dst))
    s_dst = dst[okey_e]
    s_chunk = e_chunk[okey_e]
    s_rel = e_rel[okey_e]
    key = s_dst * NCHUNK + s_chunk
    run_start = np.searchsorted(key, key)
    erank = np.arange(E) - run_start
    assert (erank < ghat[s_dst]).all()

    A = np.full((NC * G, NCHUNK, TJ), ZIDX, dtype=np.int16)
    flat_pos = (node_cg[s_dst] * NCHUNK + s_chunk) * TJ \
        + prefixB[node_slot[s_dst]] + erank
    A.reshape(-1)[flat_pos] = s_rel.astype(np.int16)

    parts = []
    for (lo, hi, raw, padded) in segs:
        blk = A[:, :, prefixB[lo]:prefixB[hi]]
        if padded > raw:
            padblk = np.full((NC * G, NCHUNK, padded - raw), ZIDX, np.int16)
            blk = np.concatenate([blk, padblk], axis=2)
        parts.append(blk)
    AP_ = np.concatenate(parts, axis=2)

    return dict(
        deg=deg, node_cg=node_cg, node_core=node_core, node_grp=node_grp,
        node_slot=node_slot, pi=pi, tmpl=tmpl, prefixB=prefixB,
        segs=segs, runs=runs, TJ=TJ, TPJ=TPJ, JMAXP=JMAXP,
        nslots_used=nslots_used, idx=AP_, levels=levels,
    )


def make_host_data(feat, weight, src, dst):
    n_nodes = feat.shape[0]
    lay = build_layout(src, dst, n_nodes)

    deg = lay["deg"].astype(np.float64)
    degc = np.maximum(deg, 1.0).astype(np.float32)
    dh = (degc ** -0.5).astype(np.float32)
    dh2 = dh * dh
    dhinv = 1.0 / dh

    node_core = lay["node_core"]
    node_grp = lay["node_grp"]
    node_slot = lay["node_slot"]
    nodes = np.arange(n_nodes)

    def slot_tile(vals):
        t = np.zeros((NC, P, SLOTS), np.float32)
        for l in range(16):
            t[node_core, node_grp * 16 + l, node_slot] = vals
        return t

    dh2l = slot_tile(dh2)
    dhhl = slot_tile(dh)
    dhil = slot_tile(dhinv)

    g0v = feat * dh[:, None]
    g0slot = np.zeros((NC, P, SLOTS, 2), np.float32)
    g0tab = np.zeros((P, NLOC, 2), np.float16)
    for l in range(16):
        g0slot[node_core, node_grp * 16 + l, node_slot, 0] = g0v[nodes, 2 * l]
        g0slot[node_core, node_grp * 16 + l, node_slot, 1] = g0v[nodes, 2 * l + 1]
        g0tab[node_core * 16 + l, node_grp * SLOTS + node_slot, 0] = \
            g0v[nodes, 2 * l].astype(np.float16)
        g0tab[node_core * 16 + l, node_grp * SLOTS + node_slot, 1] = \
            g0v[nodes, 2 * l + 1].astype(np.float16)
    g0slot = g0slot.reshape(NC, P, SLOTS * 2)
    g0tab = g0tab.reshape(P, NLOC * 2)
    # pre-replicated round-1 chunk tables (input upload is free)
    g0rep = []
    for kc in range(NCHUNK):
        rows = g0tab[2 * kc * 16:(2 * kc + 2) * 16].reshape(2, 16, NLOC * 2)
        blk = np.concatenate([rows[0], rows[1]], axis=1)      # [16, CH*2]
        g0rep.append(np.ascontiguousarray(np.tile(blk, (G, 1))))

    idx = lay["idx"].reshape(NC, G, NCHUNK * lay["TPJ"])
    idxw = idx.reshape(NC, G, -1, 16).transpose(0, 1, 3, 2)
    idxw = np.ascontiguousarray(idxw.reshape(NC, P, -1))

    coef = np.array([[0.25, 0.5, 0.25, 0.0, 0.5, 0.25]], np.float32)
    w2 = np.ascontiguousarray(weight.reshape(1, 3).astype(np.float32))

    per_core = []
    for c in range(NC):
        pc = dict(
            g0tab=g0tab, g0slot=np.ascontiguousarray(g0slot[c]),
            idx=idxw[c], dh2l=np.ascontiguousarray(dh2l[c]),
            dhhl=np.ascontiguousarray(dhhl[c]),
            dhil=np.ascontiguousarray(dhil[c]),
            w=w2, coef=coef,
        )
        for kc in range(NCHUNK):
            pc[f"g0rep{kc}"] = g0rep[kc]
        per_core.append(pc)
    return lay, per_core


def assemble_output(lay, outs, n_nodes):
    full = np.stack(outs)                          # [NC, 128, SLOTS*2]
    full = full.reshape(NC, G, 16, SLOTS, 2)
    res = np.empty((n_nodes, D), np.float32)
    nc_, ng, ns = lay["node_core"], lay["node_grp"], lay["node_slot"]
    for l in range(16):
        res[:, 2 * l] = full[nc_, ng, l, ns, 0]
        res[:, 2 * l + 1] = full[nc_, ng, l, ns, 1]
    return res


# --------------------------------------------------------------------------
# device kernel
# --------------------------------------------------------------------------

def build_kernel(segs, runs, TPJ, JMAXP):
    TOTC = NCHUNK * TPJ // 16     # idx cols per partition
    S2 = SLOTS * 2

    nc = bacc.Bacc("TRN2", target_bir_lowering=False)
    g0rep_d = [nc.dram_tensor(f"g0rep{kc}", [P, CH * 2], F16,
                              kind="ExternalInput") for kc in range(NCHUNK)]
    g0slot_d = nc.dram_tensor("g0slot", [P, S2], F32, kind="ExternalInput")
    idx_d = nc.dram_tensor("idx", [P, TOTC], I16, kind="ExternalInput")
    dh2l_d = nc.dram_tensor("dh2l", [P, SLOTS], F32, kind="ExternalInput")
    dhhl_d = nc.dram_tensor("dhhl", [P, SLOTS], F32, kind="ExternalInput")
    dhil_d = nc.dram_tensor("dhil", [P, SLOTS], F32, kind="ExternalInput")
    w_d = nc.dram_tensor("w", [1, 3], F32, kind="ExternalInput")
    coef_d = nc.dram_tensor("coef", [1, 6], F32, kind="ExternalInput")
    out_d = nc.dram_tensor("out", [P, S2], F32, kind="ExternalOutput")

    seg_off = [0]
    for s in segs:
        seg_off.append(seg_off[-1] + s[3])

    with tile.TileContext(nc) as tc:
        with (
            tc.tile_pool(name="dram", bufs=1, space="DRAM") as dramp,
            tc.tile_pool(name="persist", bufs=1) as persist,
            tc.tile_pool(name="tblp", bufs=2) as tblp,
            tc.tile_pool(name="slabp", bufs=3) as slabp,
        ):
            gtabs = [dramp.tile([P, NLOC * 2], F16, name=f"gtab{k}",
                                addr_space="Shared")
                     for k in range(2)]
            bounces = [dramp.tile([16, NLOC * 2], F16, name=f"bnc{k}")
                       for k in range(2)]

            idxt = persist.tile([P, TOTC], I16, name="idxt")
            nc.sync.dma_start(out=idxt[:], in_=idx_d[:])
            dh2l = persist.tile([P, SLOTS], F32, name="dh2l")
            dhhl = persist.tile([P, SLOTS], F32, name="dhhl")
            dhil = persist.tile([P, SLOTS], F32, name="dhil")
            nc.sync.dma_start(out=dh2l[:], in_=dh2l_d[:])
            nc.sync.dma_start(out=dhhl[:], in_=dhhl_d[:])
            nc.sync.dma_start(out=dhil[:], in_=dhil_d[:])

            # scalars s1, s2
            wt = persist.tile([1, 3], F32, name="wt")
            coefs = persist.tile([1, 6], F32, name="coefs")
            nc.sync.dma_start(out=wt[:], in_=w_d[:])
            nc.sync.dma_start(out=coefs[:], in_=coef_d[:])
            wr = persist.tile([1, 3], F32, name="wr")
            nc.vector.tensor_scalar(out=wr[:], in0=wt[:], scalar1=0.0,
                                    scalar2=None, op0=mybir.AluOpType.max)
            sprod = persist.tile([1, 6], F32, name="sprod")
            nc.vector.tensor_tensor(out=sprod[:, 0:3], in0=wr[:],
                                    in1=coefs[:, 0:3], op=mybir.AluOpType.mult)
            nc.vector.tensor_tensor(out=sprod[:, 3:6], in0=wr[:],
                                    in1=coefs[:, 3:6], op=mybir.AluOpType.mult)
            svals = persist.tile([1, 2], F32, name="svals")
            nc.vector.tensor_reduce(out=svals[:, 0:1], in_=sprod[:, 0:3],
                                    axis=mybir.AxisListType.X,
                                    op=mybir.AluOpType.add)
            nc.vector.tensor_reduce(out=svals[:, 1:2], in_=sprod[:, 3:6],
                                    axis=mybir.AxisListType.X,
                                    op=mybir.AluOpType.add)
            sbc = persist.tile([P, 2], F32, name="sbc")
            nc.gpsimd.partition_broadcast(sbc[:], svals[:])
            # fold s1/s2 into the final-combine scale tiles up front so the
            # sliced final round needs no tensor_scalar (2-port) ops
            nc.vector.tensor_scalar(out=dhil[:], in0=dhil[:],
                                    scalar1=sbc[:, 0:1], scalar2=None,
                                    op0=mybir.AluOpType.mult)
            nc.vector.tensor_scalar(out=dhhl[:], in0=dhhl[:],
                                    scalar1=sbc[:, 1:2], scalar2=None,
                                    op0=mybir.AluOpType.mult)

            zt = persist.tile([P, 1], F32, name="zt")
            nc.gpsimd.memset(zt[:], 0.0)

            gA = persist.tile([P, S2], F32, name="gA")
            gB = persist.tile([P, S2], F32, name="gB")
            nc.sync.dma_start(out=gA[:], in_=g0slot_d[:])

            partials = [persist.tile([P, S2], F32, name=f"part{kc}")
                        for kc in range(NCHUNK)]
            for t in partials:
                nc.gpsimd.memset(t[:], 0.0)

            agg = persist.tile([P, S2], F32, name="agg")
            ut = persist.tile([P, S2], F32, name="ut")
            tt = persist.tile([P, S2], F32, name="tt")
            shipt = [persist.tile([P, S2], F16, name=f"shipt{i}")
                     for i in range(2)]

            for k in (1, 2, 3):
                slab0 = None
                for kc in range(NCHUNK):
                    tbl = tblp.tile([P, CH * 2], F16, tag="tbl",
                                    name=f"tbl{k}_{kc}")
                    if kc == 1 and slab0 is not None:
                        # order-pin: chunk-1 loads WAW on this corner, which
                        # depends on the chunk-0 first gather — stops the
                        # scheduler enqueueing c1 loads before that gather's
                        # sem waits (which use cumulative lane thresholds)
                        nc.vector.tensor_tensor(
                            out=tbl[:, 0:2], in0=slab0[:, 0:2],
                            in1=slab0[:, 0:2], op=mybir.AluOpType.mult)
                    if k == 1:
                        # round 1: host pre-replicated table, one flat DMA
                        nc.sync.dma_start(out=tbl[:], in_=g0rep_d[kc][:])
                    else:
                        tabsrc = gtabs[k - 2]
                        src32 = tabsrc[2 * kc * 16:(2 * kc + 2) * 16, :] \
                            .rearrange("(h l) f -> l h f", h=2)
                        for g in range(G):
                            nc.sync.dma_start(
                                out=tbl[g * 16:(g + 1) * 16, :].rearrange(
                                    "l (h f) -> l h f", h=2),
                                in_=src32)
                    for si, (lo, hi, raw, padded) in enumerate(segs):
                        slab = slabp.tile([P, JMAXP * 2], F16, tag="slab",
                                          name=f"slab{k}_{kc}_{si}")
                        if kc == 0 and si == 0:
                            slab0 = slab
                        base16 = (kc * TPJ + seg_off[si]) // 16
                        nc.gpsimd.ap_gather(
                            out_ap=slab[:, 0:padded * 2].rearrange(
                                "p (n d) -> p n d", d=2),
                            in_ap=tbl[:].rearrange("p (n d) -> p n d", d=2),
                            idxs_ap=idxt[:, base16:base16 + padded // 16],
                            channels=P, num_elems=CH, d=2, num_idxs=padded)
                        for (si2, off, R, b, s0) in runs:
                            if si2 != si:
                                continue
                            nc.vector.tensor_reduce(
                                out=partials[kc][:, s0 * 2:(s0 + R) * 2]
                                .rearrange("p (r j) -> p r j", j=2),
                                in_=slab[:, off * 2:(off + R * b) * 2]
                                .rearrange("p (r b j) -> p r j b", b=b, j=2),
                                axis=mybir.AxisListType.X,
                                op=mybir.AluOpType.add)
                        if kc == NCHUNK - 1:
                            # slot-sliced round tail overlaps the remaining
                            # gathers of the last chunk
                            gold = gA if k != 2 else gB
                            gnew = gB if k == 1 else gA
                            lo2 = lo * 2
                            hi2 = hi * 2 if si < len(segs) - 1 else S2
                            ns = (hi2 - lo2) // 2
                            nc.vector.tensor_tensor(
                                out=agg[:, lo2:hi2],
                                in0=partials[0][:, lo2:hi2],
                                in1=partials[1][:, lo2:hi2],
                                op=mybir.AluOpType.add)
                            nc.vector.tensor_tensor(
                                out=agg[:, lo2:hi2], in0=agg[:, lo2:hi2],
                                in1=partials[2][:, lo2:hi2],
                                op=mybir.AluOpType.add)
                            nc.vector.tensor_tensor(
                                out=agg[:, lo2:hi2], in0=agg[:, lo2:hi2],
                                in1=partials[3][:, lo2:hi2],
                                op=mybir.AluOpType.add)
                            if k < 3:
                                nc.vector.tensor_tensor(
                                    out=tt[:, lo2:hi2].rearrange(
                                        "p (s j) -> p s j", j=2),
                                    in0=agg[:, lo2:hi2].rearrange(
                                        "p (s j) -> p s j", j=2),
                                    in1=dh2l[:, lo2 // 2:hi2 // 2]
                                    .to_broadcast([P, ns, 2]),
                                    op=mybir.AluOpType.mult)
                                nc.vector.tensor_tensor(
                                    out=gnew[:, lo2:hi2], in0=tt[:, lo2:hi2],
                                    in1=gold[:, lo2:hi2],
                                    op=mybir.AluOpType.add)
                                sh = shipt[k - 1]
                                nc.vector.tensor_tensor(
                                    out=sh[:, lo2:hi2], in0=gnew[:, lo2:hi2],
                                    in1=zt[:].to_broadcast([P, hi2 - lo2]),
                                    op=mybir.AluOpType.add)
                                for g in range(G):
                                    nc.sync.dma_start(
                                        out=bounces[k - 1][
                                            :, g * S2 + lo2:g * S2 + hi2],
                                        in_=sh[g * 16:(g + 1) * 16, lo2:hi2])
                            else:
                                # out = s1*dhil*g2 - s2*dhhl*agg (s1, s2
                                # pre-folded into dhil/dhhl)
                                nc.vector.tensor_tensor(
                                    out=ut[:, lo2:hi2].rearrange(
                                        "p (s j) -> p s j", j=2),
                                    in0=gold[:, lo2:hi2].rearrange(
                                        "p (s j) -> p s j", j=2),
                                    in1=dhil[:, lo2 // 2:hi2 // 2]
                                    .to_broadcast([P, ns, 2]),
                                    op=mybir.AluOpType.mult)
                                nc.vector.tensor_tensor(
                                    out=tt[:, lo2:hi2].rearrange(
                                        "p (s j) -> p s j", j=2),
                                    in0=agg[:, lo2:hi2].rearrange(
                                        "p (s j) -> p s j", j=2),
                                    in1=dhhl[:, lo2 // 2:hi2 // 2]
                                    .to_broadcast([P, ns, 2]),
                                    op=mybir.AluOpType.mult)
                                nc.vector.tensor_tensor(
                                    out=ut[:, lo2:hi2], in0=ut[:, lo2:hi2],
                                    in1=tt[:, lo2:hi2],
                                    op=mybir.AluOpType.subtract)
                                nc.sync.dma_start(out=out_d[:, lo2:hi2],
                                                  in_=ut[:, lo2:hi2])

                if k < 3:
                    nc.gpsimd.collective_compute(
                        "AllGather", mybir.AluOpType.bypass,
                        replica_groups=[list(range(NC))],
                        ins=[bounces[k - 1].opt()], outs=[gtabs[k - 1].opt()])
    nc.compile()
    return nc


_CACHE = {}


def kernel(feat, weight, src, dst):
    feat = np.ascontiguousarray(np.asarray(feat, dtype=np.float32))
    weight = np.ascontiguousarray(np.asarray(weight, dtype=np.float32))
    src64 = np.asarray(src).astype(np.int64)
    dst64 = np.asarray(dst).astype(np.int64)
    n_nodes = feat.shape[0]

    lay, per_core = make_host_data(feat, weight, src64, dst64)
    key = (tuple(lay["segs"]), tuple(lay["runs"]), lay["TPJ"], lay["JMAXP"])
    if key not in _CACHE:
        _CACHE[key] = build_kernel(lay["segs"], lay["runs"], lay["TPJ"],
                                   lay["JMAXP"])
    nc = _CACHE[key]

    keys = ["g0slot", "idx", "dh2l", "dhhl", "dhil", "w", "coef"] + \
        [f"g0rep{kc}" for kc in range(NCHUNK)]
    in_maps = [{k: pc[k] for k in keys} for pc in per_core]
    res = bass_utils.run_bass_kernel_spmd(nc, in_maps, core_ids=list(range(NC)))
    outs = [res.results[c]["out"] for c in range(NC)]
    return assemble_output(lay, outs, n_nodes)


# revision 52
# speedup vs baseline: 1.1775x; 1.0010x over previous
"""BernConv (K=2) GNN message passing on 8 Trainium2 NeuronCores.

Self-contained kernel: kernel(**inputs) -> np.ndarray [N, 32] float32.

Strategy (v2, ap_gather): keep the whole fp16 feature table SBUF-resident
per source-chunk and gather src rows with the GPSIMD `ap_gather` SBUF
gather (the dma_gather/HBM path is descriptor-rate bound at ~60 GB/s
effective for 256B random reads).

Layout: nodes dealt (by degree) over 8 cores x 8 GPSIMD groups x SLOTS
slots; feature dim pairs (2l, 2l+1) live on partition lane l of each
16-partition group, i.e. every group holds all 32 dims at d=2 fp16 per
node. The gather table [128, NLOC*2] fp16 (row c*16+l = core c, lane l)
is AllGathered between rounds; each round every core streams the table
in NCHUNK=4 source chunks into a [128, CH*2] SBUF tile (8 group
replicas) and runs per-group ap_gathers + bucketed DVE segment reduces:
    g0 = dh*feat;  g_k = g_{k-1} + dh^2 * agg(g_{k-1})   (k = 1, 2)
    out = s1 * dh^-1 * g2 - s2 * dh * agg(g2)
"""
import sys
sys.path.insert(0, "/opt/trn_rl_repo")

import numpy as np
import concourse.bacc as bacc
import concourse.mybir as mybir
import concourse.tile as tile
from concourse import bass_utils

NC = 8
G = 8
P = 128
D = 32
SLOTS = 784              # last slot (783) reserved as guaranteed-zero
NLOC = G * SLOTS         # 6272
NPAD = NC * NLOC         # 50176
NCHUNK = 4
CH = NPAD // NCHUNK      # 12544
ZIDX = CH - 1            # rel row of core 2k+1, g=7, s=783 -> reserved zero
JMAX_RAW = 2560          # target segment size (pre %16 pad)

F32 = mybir.dt.float32
F16 = mybir.dt.float16
I16 = mybir.dt.int16


# --------------------------------------------------------------------------
# host-side layout
# --------------------------------------------------------------------------

def choose_levels(req, max_levels=14):
    """Bucket levels minimizing total padded degree (DP on req histogram)."""
    Lmax = int(req.max())
    hist = np.bincount(req, minlength=Lmax + 1).astype(np.int64)
    cnt_le = hist.cumsum()
    INF = float("inf")

    def interval_cost(a, b):
        return int(cnt_le[b] - cnt_le[a]) * b

    f = np.full((max_levels + 1, Lmax + 1), INF)
    prev = np.zeros((max_levels + 1, Lmax + 1), dtype=np.int64)
    f[0, 0] = 0.0
    for m in range(1, max_levels + 1):
        for b in range(1, Lmax + 1):
            best, besta = INF, 0
            for a in range(0, b):
                if f[m - 1, a] == INF:
                    continue
                c = f[m - 1, a] + interval_cost(a, b)
                if c < best:
                    best, besta = c, a
            f[m, b] = best
            prev[m, b] = besta
    m_best = int(np.argmin(f[:, Lmax]))
    levels = []
    b, m = Lmax, m_best
    while b > 0:
        levels.append(b)
        b = int(prev[m, b])
        m -= 1
    return np.array(sorted(levels), dtype=np.int64)


def balance_assign(src, dst, deg, n_nodes):
    """Assign nodes to the 64 (c,g) groups, batch-dealing by degree (keeps
    group sizes and degree profiles aligned) while greedily choosing each
    node's CHUNK to flatten its out-neighbors' per-chunk in-edge counts
    (cuts the bucket padding that is driven by max-over-chunks)."""
    order = np.argsort(-deg, kind="stable")
    # out-adjacency sorted by src
    eo = np.argsort(src, kind="stable")
    s_src = src[eo]
    s_dst = dst[eo]
    out_start = np.searchsorted(s_src, np.arange(n_nodes))
    out_end = np.searchsorted(s_src, np.arange(n_nodes), side="right")

    cnt = np.zeros((n_nodes, NCHUNK), dtype=np.int32)
    curmax = np.zeros(n_nodes, dtype=np.int32)
    node_cg = np.full(n_nodes, -1, dtype=np.int64)
    gpc = G * (NC // NCHUNK)          # groups per chunk (16)
    nb = (n_nodes + 63) // 64

    def sweep():
        # rotate group fill order per batch so profiles stay aligned
        for b in range(nb):
            batch = order[b * 64:(b + 1) * 64]
            nbrs = [s_dst[out_start[i]:out_end[i]] for i in batch]
            # remove current contribution (refinement passes)
            for j, i in enumerate(batch):
                if node_cg[i] >= 0:
                    k_old = (node_cg[i] // G) // (NC // NCHUNK)
                    chunk_total[k_old] -= 1
                    if len(nbrs[j]):
                        np.add.at(cnt, (nbrs[j], k_old), -1)
                        curmax[nbrs[j]] = cnt[nbrs[j]].max(axis=1)
            costs = np.zeros((len(batch), NCHUNK), dtype=np.int64)
            for j, nb_j in enumerate(nbrs):
                if len(nb_j):
                    m = curmax[nb_j]
                    c = cnt[nb_j]
                    costs[j] = (2 * np.maximum(c + 1 - m[:, None], 0)
                                + (c + 1 == m[:, None])).sum(axis=0)
            # soft per-batch quota (2x the fair share) + hard global cap
            cap = np.full(NCHUNK, 2 * gpc, dtype=np.int64)
            slot_in_chunk = np.zeros(NCHUNK, dtype=np.int64)
            sc = np.sort(costs, axis=1)
            regret = sc[:, 1] - sc[:, 0] if NCHUNK > 1 else sc[:, 0]
            for j in np.argsort(-regret):
                ks = np.argsort(costs[j], kind="stable")
                k = next(k for k in ks
                         if cap[k] > 0 and chunk_total[k] < CHCAP)
                cap[k] -= 1
                chunk_total[k] += 1
                i = batch[j]
                g_off = (b + slot_in_chunk[k]) % gpc
                node_cg[i] = (k * (NC // NCHUNK) + g_off // G) * G + (g_off % G)
                slot_in_chunk[k] += 1
                nb_j = nbrs[j]
                if len(nb_j):
                    np.add.at(cnt, (nb_j, k), 1)
                    curmax[nb_j] = np.maximum(curmax[nb_j], cnt[nb_j, k])

    CHCAP = gpc * (SLOTS - 1) - 16
    chunk_total = np.zeros(NCHUNK, dtype=np.int64)
    for _ in range(5):
        sweep()
    return node_cg


def build_layout(src, dst, n_nodes):
    E = src.shape[0]
    deg = np.bincount(dst, minlength=n_nodes).astype(np.int64)

    node_cg = balance_assign(src, dst, deg, n_nodes)
    node_core = node_cg // G
    node_grp = node_cg % G
    node_chunk = node_core // (NC // NCHUNK)   # chunk of a node as src

    cnt = np.zeros((n_nodes, NCHUNK), dtype=np.int64)
    np.add.at(cnt, (dst, node_chunk[src]), 1)
    req = np.maximum(cnt.max(axis=1), 1)

    levels = choose_levels(req)
    ghat = levels[np.searchsorted(levels, req)]

    # re-deal within each chunk by ghat rank: the 16 groups of a chunk get
    # exactly aligned level profiles, so the shared template (max over all
    # 64 groups) is just the elementwise max of the 4 chunk profiles.
    gpc = G * (NC // NCHUNK)
    node_core = np.empty(n_nodes, dtype=np.int64)
    node_grp = np.empty(n_nodes, dtype=np.int64)
    node_slot = np.empty(n_nodes, dtype=np.int64)
    tmpl = np.zeros(SLOTS - 1, dtype=np.int64)
    for kchunk in range(NCHUNK):
        nk = np.where(node_chunk == kchunk)[0]
        nk = nk[np.lexsort((nk, -ghat[nk]))]
        r = np.arange(len(nk))
        go = r % gpc
        node_core[nk] = 2 * kchunk + go // G
        node_grp[nk] = go % G
        node_slot[nk] = r // gpc
        prof = ghat[nk][::gpc]            # rank-0-of-each-slot = profile max
        assert len(prof) <= SLOTS - 1, len(prof)
        tmpl[: len(prof)] = np.maximum(tmpl[: len(prof)], prof)
    node_cg = node_core * G + node_grp
    nslots_used = int((tmpl > 0).sum())
    tmpl = tmpl[:nslots_used]
    prefixB = np.concatenate([[0], np.cumsum(tmpl)])
    TJ = int(prefixB[-1])

    # segments: greedy cut at slot boundaries, each <= JMAX_RAW raw idxs
    seg_bounds = [0]
    for s in range(nslots_used):
        if prefixB[s + 1] - prefixB[seg_bounds[-1]] > JMAX_RAW:
            seg_bounds.append(s)
    seg_bounds.append(nslots_used)
    segs = []
    for i in range(len(seg_bounds) - 1):
        lo, hi = seg_bounds[i], seg_bounds[i + 1]
        raw = int(prefixB[hi] - prefixB[lo])
        # %32 (not %16): keeps every idx-slice base 4-byte aligned — the
        # gather ucode reads idxs as 32-bit words and a 2-byte-misaligned
        # AP base corrupts every 4th word's high half.
        pad = (-raw) % 32
        segs.append((lo, hi, raw, raw + pad))
    JMAXP = max(s[3] for s in segs)
    TPJ = sum(s[3] for s in segs)

    # runs of equal level inside each segment
    runs = []
    for si, (lo, hi, raw, padded) in enumerate(segs):
        s = lo
        off = 0
        while s < hi:
            b = int(tmpl[s])
            e = s
            while e < hi and tmpl[e] == b:
                e += 1
            runs.append((si, off, e - s, b, s))
            off += (e - s) * b
            s = e

    # fill per (c,g,chunk) idx arrays with ZIDX pads
    pi = node_core * NLOC + node_grp * SLOTS + node_slot
    src_row = pi[src]
    e_chunk = src_row // CH
    e_rel = src_row - e_chunk * CH

    okey_e = np.lexsort((e_rel, e_chunk, dst))
    s_dst = dst[okey_e]
    s_chunk = e_chunk[okey_e]
    s_rel = e_rel[okey_e]
    key = s_dst * NCHUNK + s_chunk
    run_start = np.searchsorted(key, key)
    erank = np.arange(E) - run_start
    assert (erank < ghat[s_dst]).all()

    A = np.full((NC * G, NCHUNK, TJ), ZIDX, dtype=np.int16)
    flat_pos = (node_cg[s_dst] * NCHUNK + s_chunk) * TJ \
        + prefixB[node_slot[s_dst]] + erank
    A.reshape(-1)[flat_pos] = s_rel.astype(np.int16)

    parts = []
    for (lo, hi, raw, padded) in segs:
        blk = A[:, :, prefixB[lo]:prefixB[hi]]
        if padded > raw:
            padblk = np.full((NC * G, NCHUNK, padded - raw), ZIDX, np.int16)
            blk = np.concatenate([blk, padblk], axis=2)
        parts.append(blk)
    AP_ = np.concatenate(parts, axis=2)

    return dict(
        deg=deg, node_cg=node_cg, node_core=node_core, node_grp=node_grp,
        node_slot=node_slot, pi=pi, tmpl=tmpl, prefixB=prefixB,
        segs=segs, runs=runs, TJ=TJ, TPJ=TPJ, JMAXP=JMAXP,
        nslots_used=nslots_used, idx=AP_, levels=levels,
    )


def make_host_data(feat, weight, src, dst):
    n_nodes = feat.shape[0]
    lay = build_layout(src, dst, n_nodes)

    deg = lay["deg"].astype(np.float64)
    degc = np.maximum(deg, 1.0).astype(np.float32)
    dh = (degc ** -0.5).astype(np.float32)
    dh2 = dh * dh
    dhinv = 1.0 / dh

    node_core = lay["node_core"]
    node_grp = lay["node_grp"]
    node_slot = lay["node_slot"]
    nodes = np.arange(n_nodes)

    def slot_tile(vals):
        t = np.zeros((NC, P, SLOTS), np.float32)
        for l in range(16):
            t[node_core, node_grp * 16 + l, node_slot] = vals
        return t

    dh2l = slot_tile(dh2)
    dhhl = slot_tile(dh)
    dhil = slot_tile(dhinv)

    g0v = feat * dh[:, None]
    g0slot = np.zeros((NC, P, SLOTS, 2), np.float32)
    g0tab = np.zeros((P, NLOC, 2), np.float16)
    for l in range(16):
        g0slot[node_core, node_grp * 16 + l, node_slot, 0] = g0v[nodes, 2 * l]
        g0slot[node_core, node_grp * 16 + l, node_slot, 1] = g0v[nodes, 2 * l + 1]
        g0tab[node_core * 16 + l, node_grp * SLOTS + node_slot, 0] = \
            g0v[nodes, 2 * l].astype(np.float16)
        g0tab[node_core * 16 + l, node_grp * SLOTS + node_slot, 1] = \
            g0v[nodes, 2 * l + 1].astype(np.float16)
    g0slot = g0slot.reshape(NC, P, SLOTS * 2)
    g0tab = g0tab.reshape(P, NLOC * 2)
    # pre-replicated round-1 chunk tables (input upload is free)
    g0rep = []
    for kc in range(NCHUNK):
        rows = g0tab[2 * kc * 16:(2 * kc + 2) * 16].reshape(2, 16, NLOC * 2)
        blk = np.concatenate([rows[0], rows[1]], axis=1)      # [16, CH*2]
        g0rep.append(np.ascontiguousarray(np.tile(blk, (G, 1))))

    idx = lay["idx"].reshape(NC, G, NCHUNK * lay["TPJ"])
    idxw = idx.reshape(NC, G, -1, 16).transpose(0, 1, 3, 2)
    idxw = np.ascontiguousarray(idxw.reshape(NC, P, -1))

    coef = np.array([[0.25, 0.5, 0.25, 0.0, 0.5, 0.25]], np.float32)
    w2 = np.ascontiguousarray(weight.reshape(1, 3).astype(np.float32))

    per_core = []
    for c in range(NC):
        pc = dict(
            g0tab=g0tab, g0slot=np.ascontiguousarray(g0slot[c]),
            idx=idxw[c], dh2l=np.ascontiguousarray(dh2l[c]),
            dhhl=np.ascontiguousarray(dhhl[c]),
            dhil=np.ascontiguousarray(dhil[c]),
            w=w2, coef=coef,
        )
        for kc in range(NCHUNK):
            pc[f"g0rep{kc}"] = g0rep[kc]
        per_core.append(pc)
    return lay, per_core


def assemble_output(lay, outs, n_nodes):
    full = np.stack(outs)                          # [NC, 128, SLOTS*2]
    full = full.reshape(NC, G, 16, SLOTS, 2)
    res = np.empty((n_nodes, D), np.float32)
    nc_, ng, ns = lay["node_core"], lay["node_grp"], lay["node_slot"]
    for l in range(16):
        res[:, 2 * l] = full[nc_, ng, l, ns, 0]
        res[:, 2 * l + 1] = full[nc_, ng, l, ns, 1]
    return res


# --------------------------------------------------------------------------
# device kernel
# --------------------------------------------------------------------------

def build_kernel(segs, runs, TPJ, JMAXP):
    TOTC = NCHUNK * TPJ // 16     # idx cols per partition
    S2 = SLOTS * 2

    nc = bacc.Bacc("TRN2", target_bir_lowering=False)
    g0rep_d = [nc.dram_tensor(f"g0rep{kc}", [P, CH * 2], F16,
                              kind="ExternalInput") for kc in range(NCHUNK)]
    g0slot_d = nc.dram_tensor("g0slot", [P, S2], F32, kind="ExternalInput")
    idx_d = nc.dram_tensor("idx", [P, TOTC], I16, kind="ExternalInput")
    dh2l_d = nc.dram_tensor("dh2l", [P, SLOTS], F32, kind="ExternalInput")
    dhhl_d = nc.dram_tensor("dhhl", [P, SLOTS], F32, kind="ExternalInput")
    dhil_d = nc.dram_tensor("dhil", [P, SLOTS], F32, kind="ExternalInput")
    w_d = nc.dram_tensor("w", [1, 3], F32, kind="ExternalInput")
    coef_d = nc.dram_tensor("coef", [1, 6], F32, kind="ExternalInput")
    out_d = nc.dram_tensor("out", [P, S2], F32, kind="ExternalOutput")

    seg_off = [0]
    for s in segs:
        seg_off.append(seg_off[-1] + s[3])

    with tile.TileContext(nc) as tc:
        with (
            tc.tile_pool(name="dram", bufs=1, space="DRAM") as dramp,
            tc.tile_pool(name="persist", bufs=1) as persist,
            tc.tile_pool(name="tblp", bufs=2) as tblp,
            tc.tile_pool(name="slabp", bufs=3) as slabp,
        ):
            gtabs = [dramp.tile([P, NLOC * 2], F16, name=f"gtab{k}",
                                addr_space="Shared")
                     for k in range(2)]
            bounces = [dramp.tile([16, NLOC * 2], F16, name=f"bnc{k}")
                       for k in range(2)]

            idxt = persist.tile([P, TOTC], I16, name="idxt")
            nc.sync.dma_start(out=idxt[:], in_=idx_d[:])
            dh2l = persist.tile([P, SLOTS], F32, name="dh2l")
            dhhl = persist.tile([P, SLOTS], F32, name="dhhl")
            dhil = persist.tile([P, SLOTS], F32, name="dhil")
            nc.sync.dma_start(out=dh2l[:], in_=dh2l_d[:])
            nc.sync.dma_start(out=dhhl[:], in_=dhhl_d[:])
            nc.sync.dma_start(out=dhil[:], in_=dhil_d[:])

            # scalars s1, s2
            wt = persist.tile([1, 3], F32, name="wt")
            coefs = persist.tile([1, 6], F32, name="coefs")
            nc.sync.dma_start(out=wt[:], in_=w_d[:])
            nc.sync.dma_start(out=coefs[:], in_=coef_d[:])
            wr = persist.tile([1, 3], F32, name="wr")
            nc.vector.tensor_scalar(out=wr[:], in0=wt[:], scalar1=0.0,
                                    scalar2=None, op0=mybir.AluOpType.max)
            sprod = persist.tile([1, 6], F32, name="sprod")
            nc.vector.tensor_tensor(out=sprod[:, 0:3], in0=wr[:],
                                    in1=coefs[:, 0:3], op=mybir.AluOpType.mult)
            nc.vector.tensor_tensor(out=sprod[:, 3:6], in0=wr[:],
                                    in1=coefs[:, 3:6], op=mybir.AluOpType.mult)
            svals = persist.tile([1, 2], F32, name="svals")
            nc.vector.tensor_reduce(out=svals[:, 0:1], in_=sprod[:, 0:3],
                                    axis=mybir.AxisListType.X,
                                    op=mybir.AluOpType.add)
            nc.vector.tensor_reduce(out=svals[:, 1:2], in_=sprod[:, 3:6],
                                    axis=mybir.AxisListType.X,
                                    op=mybir.AluOpType.add)
            sbc = persist.tile([P, 2], F32, name="sbc")
            nc.gpsimd.partition_broadcast(sbc[:], svals[:])
            # fold s1/s2 into the final-combine scale tiles up front so the
            # sliced final round needs no tensor_scalar (2-port) ops
            nc.vector.tensor_scalar(out=dhil[:], in0=dhil[:],
                                    scalar1=sbc[:, 0:1], scalar2=None,
                                    op0=mybir.AluOpType.mult)
            nc.vector.tensor_scalar(out=dhhl[:], in0=dhhl[:],
                                    scalar1=sbc[:, 1:2], scalar2=None,
                                    op0=mybir.AluOpType.mult)

            zt = persist.tile([P, 1], F32, name="zt")
            nc.gpsimd.memset(zt[:], 0.0)

            gA = persist.tile([P, S2], F32, name="gA")
            gB = persist.tile([P, S2], F32, name="gB")
            nc.sync.dma_start(out=gA[:], in_=g0slot_d[:])

            partials = [persist.tile([P, S2], F32, name=f"part{kc}")
                        for kc in range(NCHUNK)]
            for t in partials:
                nc.gpsimd.memset(t[:], 0.0)

            agg = persist.tile([P, S2], F32, name="agg")
            ut = persist.tile([P, S2], F32, name="ut")
            tt = persist.tile([P, S2], F32, name="tt")
            shipt = [persist.tile([P, S2], F16, name=f"shipt{i}")
                     for i in range(2)]

            for k in (1, 2, 3):
                slab0 = None
                for kc in range(NCHUNK):
                    tbl = tblp.tile([P, CH * 2], F16, tag="tbl",
                                    name=f"tbl{k}_{kc}")
                    if kc == 1 and slab0 is not None:
                        # order-pin: chunk-1 loads WAW on this corner, which
                        # depends on the chunk-0 first gather — stops the
                        # scheduler enqueueing c1 loads before that gather's
                        # sem waits (which use cumulative lane thresholds)
                        nc.vector.tensor_tensor(
                            out=tbl[:, 0:2], in0=slab0[:, 0:2],
                            in1=slab0[:, 0:2], op=mybir.AluOpType.mult)
                    if k == 1:
                        # round 1: host pre-replicated table, one flat DMA
                        nc.sync.dma_start(out=tbl[:], in_=g0rep_d[kc][:])
                    else:
                        tabsrc = gtabs[k - 2]
                        src32 = tabsrc[2 * kc * 16:(2 * kc + 2) * 16, :] \
                            .rearrange("(h l) f -> l h f", h=2)
                        for g in range(G):
                            nc.sync.dma_start(
                                out=tbl[g * 16:(g + 1) * 16, :].rearrange(
                                    "l (h f) -> l h f", h=2),
                                in_=src32)
                    for si, (lo, hi, raw, padded) in enumerate(segs):
                        slab = slabp.tile([P, JMAXP * 2], F16, tag="slab",
                                          name=f"slab{k}_{kc}_{si}")
                        if kc == 0 and si == 0:
                            slab0 = slab
                        base16 = (kc * TPJ + seg_off[si]) // 16
                        nc.gpsimd.ap_gather(
                            out_ap=slab[:, 0:padded * 2].rearrange(
                                "p (n d) -> p n d", d=2),
                            in_ap=tbl[:].rearrange("p (n d) -> p n d", d=2),
                            idxs_ap=idxt[:, base16:base16 + padded // 16],
                            channels=P, num_elems=CH, d=2, num_idxs=padded)
                        for (si2, off, R, b, s0) in runs:
                            if si2 != si:
                                continue
                            nc.vector.tensor_reduce(
                                out=partials[kc][:, s0 * 2:(s0 + R) * 2]
                                .rearrange("p (r j) -> p r j", j=2),
                                in_=slab[:, off * 2:(off + R * b) * 2]
                                .rearrange("p (r b j) -> p r j b", b=b, j=2),
                                axis=mybir.AxisListType.X,
                                op=mybir.AluOpType.add)
                        if kc == NCHUNK - 1:
                            # slot-sliced round tail overlaps the remaining
                            # gathers of the last chunk
                            gold = gA if k != 2 else gB
                            gnew = gB if k == 1 else gA
                            lo2 = lo * 2
                            hi2 = hi * 2 if si < len(segs) - 1 else S2
                            ns = (hi2 - lo2) // 2
                            nc.vector.tensor_tensor(
                                out=agg[:, lo2:hi2],
                                in0=partials[0][:, lo2:hi2],
                                in1=partials[1][:, lo2:hi2],
                                op=mybir.AluOpType.add)
                            nc.vector.tensor_tensor(
                                out=agg[:, lo2:hi2], in0=agg[:, lo2:hi2],
                                in1=partials[2][:, lo2:hi2],
                                op=mybir.AluOpType.add)
                            nc.vector.tensor_tensor(
                                out=agg[:, lo2:hi2], in0=agg[:, lo2:hi2],
                                in1=partials[3][:, lo2:hi2],
                                op=mybir.AluOpType.add)
                            if k < 3:
                                nc.vector.tensor_tensor(
                                    out=tt[:, lo2:hi2].rearrange(
                                        "p (s j) -> p s j", j=2),
                                    in0=agg[:, lo2:hi2].rearrange(
                                        "p (s j) -> p s j", j=2),
                                    in1=dh2l[:, lo2 // 2:hi2 // 2]
                                    .to_broadcast([P, ns, 2]),
                                    op=mybir.AluOpType.mult)
                                nc.vector.tensor_tensor(
                                    out=gnew[:, lo2:hi2], in0=tt[:, lo2:hi2],
                                    in1=gold[:, lo2:hi2],
                                    op=mybir.AluOpType.add)
                                sh = shipt[k - 1]
                                nc.vector.tensor_tensor(
                                    out=sh[:, lo2:hi2], in0=gnew[:, lo2:hi2],
                                    in1=zt[:].to_broadcast([P, hi2 - lo2]),
                                    op=mybir.AluOpType.add)
                                for g in range(G):
                                    nc.sync.dma_start(
                                        out=bounces[k - 1][
                                            :, g * S2 + lo2:g * S2 + hi2],
                                        in_=sh[g * 16:(g + 1) * 16, lo2:hi2])
                            else:
                                # out = s1*dhil*g2 - s2*dhhl*agg (s1, s2
                                # pre-folded into dhil/dhhl)
                                nc.vector.tensor_tensor(
                                    out=ut[:, lo2:hi2].rearrange(
                                        "p (s j) -> p s j", j=2),
                                    in0=gold[:, lo2:hi2].rearrange(
                                        "p (s j) -> p s j", j=2),
                                    in1=dhil[:, lo2 // 2:hi2 // 2]
                                    .to_broadcast([P, ns, 2]),
                                    op=mybir.AluOpType.mult)
                                nc.vector.tensor_tensor(
                                    out=tt[:, lo2:hi2].rearrange(
                                        "p (s j) -> p s j", j=2),
                                    in0=agg[:, lo2:hi2].rearrange(
                                        "p (s j) -> p s j", j=2),
                                    in1=dhhl[:, lo2 // 2:hi2 // 2]
                                    .to_broadcast([P, ns, 2]),
                                    op=mybir.AluOpType.mult)
                                nc.vector.tensor_tensor(
                                    out=ut[:, lo2:hi2], in0=ut[:, lo2:hi2],
                                    in1=tt[:, lo2:hi2],
                                    op=mybir.AluOpType.subtract)
                                nc.sync.dma_start(out=out_d[:, lo2:hi2],
                                                  in_=ut[:, lo2:hi2])

                if k < 3:
                    nc.gpsimd.collective_compute(
                        "AllGather", mybir.AluOpType.bypass,
                        replica_groups=[list(range(NC))],
                        ins=[bounces[k - 1].opt()], outs=[gtabs[k - 1].opt()])
    nc.compile()
    return nc


_CACHE = {}


def kernel(feat, weight, src, dst):
    feat = np.ascontiguousarray(np.asarray(feat, dtype=np.float32))
    weight = np.ascontiguousarray(np.asarray(weight, dtype=np.float32))
    src64 = np.asarray(src).astype(np.int64)
    dst64 = np.asarray(dst).astype(np.int64)
    n_nodes = feat.shape[0]

    lay, per_core = make_host_data(feat, weight, src64, dst64)
    key = (tuple(lay["segs"]), tuple(lay["runs"]), lay["TPJ"], lay["JMAXP"])
    if key not in _CACHE:
        _CACHE[key] = build_kernel(lay["segs"], lay["runs"], lay["TPJ"],
                                   lay["JMAXP"])
    nc = _CACHE[key]

    keys = ["g0slot", "idx", "dh2l", "dhhl", "dhil", "w", "coef"] + \
        [f"g0rep{kc}" for kc in range(NCHUNK)]
    in_maps = [{k: pc[k] for k in keys} for pc in per_core]
    res = bass_utils.run_bass_kernel_spmd(nc, in_maps, core_ids=list(range(NC)))
    outs = [res.results[c]["out"] for c in range(NC)]
    return assemble_output(lay, outs, n_nodes)
